# revision 7
# baseline (speedup 1.0000x reference)
"""Trainium2 Bass kernel for the LRU (Linear Recurrent Unit) problem.

Math: the reference's log-space parallel scan is exactly the diagonal complex
linear recurrence
    u_t = gamma * (B_w @ x_t + eps)              (input projection)
    h_t = lambda * h_{t-1} + u_t                 (complex diagonal scan)
    y_t = Re(C_w @ h_t) + d_vec * x_t            (output projection)
with lambda = exp(-exp(nu_log) + i exp(theta_log)), |lambda| in [0.5, 0.95].

Device algorithm (per core, batch-parallel over b: 2 of 16 batches/core):
  - in-projection on PE:  uT[e, f] = sum_d Win[d, e] * xT[d, f]   (re, im)
  - phase rotation on DVE: v_t = e^{-i theta t} * u_t  (host cos/sin tables)
  - two real scans (tensor_tensor_scan, multiplier rho = |lambda|, fp32 state)
  - inverse rotation as 4 products A=Gre*c, B=Gim*s, C=Gre*s, D=Gim*c; the
    recombination h_re=A-B, h_im=C+D is folded into the out-projection PSUM
    accumulation (weight planes [CreT, -CreT, -CimT, -CimT]).
  - out-projection on PE + diag(d_vec) fold for the d_vec * x term.
All transposes are done host-side (numpy); the device only sees layouts it
likes: (d on partitions, f = b*T + t on the free axis). Time is processed in
4 chunks of 512 with double-buffered planes so DVE work on chunk k overlaps
PE/DMA work on chunks k-1 / k+1; scan state chains across chunks via seeds.
"""

import os
import numpy as np

T, B, D = 2048, 16, 256
NCORES = 8
BLOC = B // NCORES          # batches per core
TC = 512                    # time chunk
NCH = T // TC
F = BLOC * T                # free elems per core (f = b*T + t)
NT = 512                    # matmul free tile

_cache = {}


def _build_program():
    from contextlib import ExitStack
    import concourse.tile as tile
    from concourse import bacc, mybir

    fp32 = mybir.dt.float32
    Alu = mybir.AluOpType

    nc = bacc.Bacc("TRN2", target_bir_lowering=False, debug=False,
                   enable_asserts=False)

    xT = nc.dram_tensor("xT", [D, BLOC, T], fp32, kind="ExternalInput").ap()
    cos_d = nc.dram_tensor("cos_t", [D, T], fp32, kind="ExternalInput").ap()
    sin_d = nc.dram_tensor("sin_t", [D, T], fp32, kind="ExternalInput").ap()
    w_in = nc.dram_tensor("w_in", [2, D, D], fp32, kind="ExternalInput").ap()
    w_out = nc.dram_tensor("w_out", [4, D, D], fp32, kind="ExternalInput").ap()
    dg = nc.dram_tensor("dg", [D, 128], fp32, kind="ExternalInput").ap()
    rho = nc.dram_tensor("rho", [D, 1], fp32, kind="ExternalInput").ap()

    y_t = nc.dram_tensor("y_t", [D, F], fp32, kind="ExternalOutput").ap()
    hlast = nc.dram_tensor("hlast", [D, 4], fp32, kind="ExternalOutput").ap()

    with tile.TileContext(nc) as tc, ExitStack() as ctx:
        wpool = ctx.enter_context(tc.tile_pool(name="w", bufs=1))
        xpool = ctx.enter_context(tc.tile_pool(name="x", bufs=2))
        tpool = ctx.enter_context(tc.tile_pool(name="tabs", bufs=2))
        ppool = ctx.enter_context(tc.tile_pool(name="planes", bufs=2))
        spool = ctx.enter_context(tc.tile_pool(name="scratch", bufs=2))
        mpool = ctx.enter_context(tc.tile_pool(name="mm", bufs=4, space="PSUM"))
        ypool = ctx.enter_context(tc.tile_pool(name="ystage", bufs=4))

        # --- persistent weights -------------------------------------------
        win_t = [[wpool.tile([128, D], fp32, tag=f"win{c}{k}", name=f"win{c}{k}")
                  for k in range(2)] for c in range(2)]
        wout_t = [[wpool.tile([128, D], fp32, tag=f"wout{p}{k}", name=f"wout{p}{k}")
                   for k in range(2)] for p in range(4)]
        dg_t = [wpool.tile([128, 128], fp32, tag=f"dg{k}", name=f"dg{k}")
                for k in range(2)]
        rho_t = [wpool.tile([128, 1], fp32, tag=f"rho{k}", name=f"rho{k}")
                 for k in range(2)]
        rho_b = [wpool.tile([128, TC], fp32, tag=f"rhob{k}", name=f"rhob{k}")
                 for k in range(2)]
        seed = [[[wpool.tile([128, 1], fp32, tag=f"seed{c}{b}{k}",
                             name=f"seed{c}{b}{k}")
                  for k in range(2)] for b in range(BLOC)] for c in range(2)]
        hl_t = [wpool.tile([128, 4], fp32, tag=f"hl{k}", name=f"hl{k}")
                for k in range(2)]

        for k in range(2):
            sl = slice(128 * k, 128 * (k + 1))
            for c in range(2):
                nc.sync.dma_start(win_t[c][k][:], w_in[c, sl, :])
            for p in range(4):
                nc.sync.dma_start(wout_t[p][k][:], w_out[p, sl, :])
            nc.sync.dma_start(dg_t[k][:], dg[sl, :])
            nc.sync.dma_start(rho_t[k][:], rho[sl, :])
            nc.gpsimd.memset(rho_b[k][:], 1.0)
            nc.scalar.mul(rho_b[k][:], rho_b[k][:], rho_t[k][:])

        # --- main pipeline over time chunks --------------------------------
        for tcki in range(NCH):
            tsl = slice(TC * tcki, TC * (tcki + 1))
            xt = [xpool.tile([128, BLOC, TC], fp32, tag=f"xt{k}", name=f"xt{k}")
                  for k in range(2)]
            cs = [tpool.tile([128, TC], fp32, tag=f"cs{k}", name=f"cs{k}")
                  for k in range(2)]
            sn = [tpool.tile([128, TC], fp32, tag=f"sn{k}", name=f"sn{k}")
                  for k in range(2)]
            for k in range(2):
                sl = slice(128 * k, 128 * (k + 1))
                nc.sync.dma_start(xt[k][:], xT[sl, :, tsl])
                nc.sync.dma_start(cs[k][:], cos_d[sl, tsl])
                nc.sync.dma_start(sn[k][:], sin_d[sl, tsl])

            # u planes (become v, then postscale A/B in place) + G + C/D
            u = [[ppool.tile([128, BLOC, TC], fp32, tag=f"u{c}{k}",
                             name=f"u{c}{k}") for k in range(2)]
                 for c in range(2)]
            g = [[ppool.tile([128, BLOC, TC], fp32, tag=f"g{c}{k}",
                             name=f"g{c}{k}") for k in range(2)]
                 for c in range(2)]
            cd = [[ppool.tile([128, BLOC, TC], fp32, tag=f"cd{c}{k}",
                              name=f"cd{c}{k}") for k in range(2)]
                  for c in range(2)]

            # in-projection
            for eh in range(2):
                for c in range(2):
                    for nt in range(BLOC * TC // NT):
                        b_i, t0 = divmod(nt * NT, TC)
                        ps = mpool.tile([128, NT], fp32, tag="psA", name="psA")
                        for kh in range(2):
                            nc.tensor.matmul(
                                ps[:],
                                win_t[c][kh][:, 128 * eh:128 * (eh + 1)],
                                xt[kh][:, b_i, t0:t0 + NT],
                                start=(kh == 0), stop=(kh == 1))
                        nc.scalar.copy(u[c][eh][:, b_i, t0:t0 + NT], ps[:])

            for k in range(2):
                cb = cs[k][:, None, :].broadcast_to([128, BLOC, TC])
                sb = sn[k][:, None, :].broadcast_to([128, BLOC, TC])
                s1 = spool.tile([128, BLOC, TC], fp32, tag="s1", name="s1")
                s2 = spool.tile([128, BLOC, TC], fp32, tag="s2", name="s2")
                s3 = spool.tile([128, BLOC, TC], fp32, tag="s3", name="s3")
                ur, ui = u[0][k], u[1][k]
                # prescale: v = e^{-i theta t} u   (v into u planes)
                nc.vector.tensor_tensor(s1[:], ur[:], cb, Alu.mult)
                nc.vector.tensor_tensor(s2[:], ur[:], sb, Alu.mult)
                nc.vector.tensor_tensor(s3[:], ui[:], sb, Alu.mult)
                nc.vector.tensor_tensor(ur[:], s1[:], s3[:], Alu.add)       # v_re
                nc.vector.tensor_tensor(s1[:], ui[:], cb, Alu.mult)
                nc.vector.tensor_tensor(ui[:], s1[:], s2[:], Alu.subtract)  # v_im

                # scans (fp32 state); chain chunks via seed tiles
                for c in range(2):
                    for b_i in range(BLOC):
                        init = 0.0 if tcki == 0 else seed[c][b_i][k][:]
                        nc.vector.tensor_tensor_scan(
                            g[c][k][:, b_i, :], rho_b[k][:], u[c][k][:, b_i, :],
                            init, Alu.mult, Alu.add)
                        if tcki < NCH - 1:
                            nc.vector.tensor_copy(seed[c][b_i][k][:],
                                                  g[c][k][:, b_i, TC - 1:TC])

                # postscale products only; h_re = A - B, h_im = C + D folded
                # into the out-projection weight planes.
                gr, gi = g[0][k], g[1][k]
                nc.vector.tensor_tensor(ur[:], gr[:], cb, Alu.mult)         # A
                nc.vector.tensor_tensor(ui[:], gi[:], sb, Alu.mult)         # B
                nc.vector.tensor_tensor(cd[0][k][:], gr[:], sb, Alu.mult)   # C
                nc.vector.tensor_tensor(cd[1][k][:], gi[:], cb, Alu.mult)   # D

                if tcki == NCH - 1:
                    for b_i in range(BLOC):
                        lc = slice(TC - 1, TC)
                        nc.vector.tensor_tensor(
                            hl_t[k][:, b_i:b_i + 1], ur[:, b_i, lc],
                            ui[:, b_i, lc], Alu.subtract)
                        nc.vector.tensor_tensor(
                            hl_t[k][:, 2 + b_i:3 + b_i], cd[0][k][:, b_i, lc],
                            cd[1][k][:, b_i, lc], Alu.add)
                    nc.sync.dma_start(hlast[128 * k:128 * (k + 1), :], hl_t[k][:])

            # out-projection: y = A@Cre - B@Cre - C@Cim - D@Cim + dvec*x
            planes = [u[0], u[1], cd[0], cd[1]]
            for eh in range(2):
                for nt in range(BLOC * TC // NT):
                    b_i, t0 = divmod(nt * NT, TC)
                    ps = mpool.tile([128, NT], fp32, tag="psB", name="psB")
                    first = True
                    for p in range(4):
                        for kh in range(2):
                            nc.tensor.matmul(
                                ps[:],
                                wout_t[p][kh][:, 128 * eh:128 * (eh + 1)],
                                planes[p][kh][:, b_i, t0:t0 + NT],
                                start=first, stop=False)
                            first = False
                    nc.tensor.matmul(ps[:], dg_t[eh][:],
                                     xt[eh][:, b_i, t0:t0 + NT],
                                     start=False, stop=True)
                    ys = ypool.tile([128, NT], fp32, tag="ys", name="ys")
                    nc.scalar.copy(ys[:], ps[:])
                    fg = b_i * T + TC * tcki + t0
                    nc.sync.dma_start(
                        y_t[128 * eh:128 * (eh + 1), fg:fg + NT], ys[:])

    nc.compile()
    return nc


def _prepare(inputs):
    x = np.ascontiguousarray(inputs["x"], dtype=np.float32)
    nu_log = np.asarray(inputs["nu_log"], np.float64)
    theta_log = np.asarray(inputs["theta_log"], np.float64)
    B_w = np.asarray(inputs["B_w"], np.complex128)
    C_w = np.asarray(inputs["C_w"], np.complex128)
    d_vec = np.asarray(inputs["d_vec"], np.float64)
    gamma_log = np.asarray(inputs["gamma_log"], np.float64)

    theta = np.exp(theta_log)
    rho = np.exp(-np.exp(nu_log)).astype(np.float32).reshape(D, 1)
    gamma = np.exp(gamma_log)

    ang = np.outer(theta, np.arange(T, dtype=np.float64)) % (2 * np.pi)
    cos_t = np.ascontiguousarray(np.cos(ang), np.float32)
    sin_t = np.ascontiguousarray(np.sin(ang), np.float32)

    Win = gamma[:, None] * B_w                      # (e, d)
    w_in = np.ascontiguousarray(
        np.stack([Win.real.T, Win.imag.T]), np.float32)     # (2, d, e)
    CreT = C_w.real.T
    CimT = C_w.imag.T
    w_out = np.ascontiguousarray(
        np.stack([CreT, -CreT, -CimT, -CimT]), np.float32)  # (4, d, e)
    dgm = np.zeros((D, 128), np.float32)
    dgm[np.arange(128), np.arange(128)] = d_vec[:128]
    dgm[128 + np.arange(128), np.arange(128)] = d_vec[128:]

    shared = {"cos_t": cos_t, "sin_t": sin_t, "w_in": w_in, "w_out": w_out,
              "dg": dgm, "rho": rho}
    in_maps = []
    for c in range(NCORES):
        xs = x[:, BLOC * c:BLOC * (c + 1), :]       # (T, BLOC, D)
        xTc = np.ascontiguousarray(xs.transpose(2, 1, 0))   # (D, BLOC, T)
        m = dict(shared)
        m["xT"] = xTc
        in_maps.append(m)
    return in_maps


def kernel(**inputs):
    from concourse.bass_utils import run_bass_kernel_spmd

    if "nc" not in _cache:
        _cache["nc"] = _build_program()
    nc = _cache["nc"]

    in_maps = _prepare(inputs)
    res = run_bass_kernel_spmd(nc, in_maps, core_ids=list(range(NCORES)),
                               trace=bool(int(os.environ.get("LRU_TRACE", "0"))))
    _cache["last_result"] = res

    y = np.empty((T, B, D), np.float32)
    hT = np.empty((B, D), np.complex64)
    for c, r in enumerate(res.results):
        yt = r["y_t"]                                # (D, F) f = b*T + t
        hl = r["hlast"]                              # (D, 4)
        for b_i in range(BLOC):
            y[:, BLOC * c + b_i, :] = yt[:, b_i * T:(b_i + 1) * T].T
            hT[BLOC * c + b_i, :] = hl[:, b_i] + 1j * hl[:, 2 + b_i]
    return y, hT
